# revision 46
# baseline (speedup 1.0000x reference)
"""Trainium2 Bass kernel for nn_PfAAMLayer (SimAM-style attention layer).

Reference math (x: [b=8, c=32, t=32, h=64, w=64] fp32):
    z  = mean(x, axis=1)                      # channel-mean gate  [b,1,t,h,w]
    s  = mean(x, axis=0)                      # batch-mean gate    [1,c,t,h,w]
    mu = mean(x, axis=(2,3,4))                # per-(b,c) mean
    d2 = (x - mu)^2
    denom = 4*(sum(d2, (2,3,4))/(h*w-1) + 1e-4)
    y  = d2/denom + 0.5
    out = x * sigmoid(y) * sigmoid(y*z*s)

Sharding: split t (axis 2) across 8 cores -> z and s are core-local;
only the per-(b,c) scalars sum(x), sum(x^2) need a 512-float AllReduce.

Per-core layout: x_local viewed as [256 (b,c) rows, 16384 (t_loc,h,w) cols],
held SBUF-resident as two [128, 16384] tiles (dtype float32r so the
TensorEngine consumes them at full rate; every other engine bitcasts to
fp32).  z/s broadcasts are produced on TensorE with 0/1 averaging matrices,
per-(b,c) stats come from one bn_stats pass on VectorE, the squares and
sigmoids run on ScalarE with per-partition scale/bias, and the remaining
elementwise multiplies are split between VectorE and GPSIMD.
"""

import sys

sys.path.insert(0, "/opt/trn_rl_repo")

import numpy as np

B, C, T, H, W = 8, 32, 32, 64, 64
NCORES = 8
TLOC = T // NCORES           # 4
POS = TLOC * H * W           # 16384 positions per core
NLOC = float(POS)
ROWS = B * C                 # 256 (b,c) rows -> 2 tiles of 128
NG = float(T * H * W)        # 131072, global per-(b,c) element count
NSIMAM = float(W * H - 1)    # 4095 (reference quirk: h*w-1, not n-1)
LAM = 1e-4

SC = 4096                    # load / bn_stats chunk (free dim)
AC = 1024                    # phase-2 ACT / output chunk
FC = 512                     # PSUM / fine chunk (one PSUM bank of fp32)

_CACHE = {}


def _mats():
    """Averaging matrices: out[m,n] = sum_p M[p,m] * x[p,n].

    Mz: mean over c within the b-group (32 partitions/group).
    Ms: mean over b (rows p = b*32 + c; same c = same p mod 32); applied to
        both 128-row tiles with PSUM accumulation -> mean over all 8 b.
    """
    p = np.arange(128)
    m = np.arange(128)
    mz = np.where(p[:, None] // 32 == m[None, :] // 32, np.float32(1.0 / 32), np.float32(0.0))
    ms = np.where(p[:, None] % 32 == m[None, :] % 32, np.float32(1.0 / 8), np.float32(0.0))
    return np.ascontiguousarray(mz, np.float32), np.ascontiguousarray(ms, np.float32)


def _build(with_collective=True):
    import concourse.bass as bass
    import concourse.bacc as bacc
    import concourse.tile as tile
    import concourse.mybir as mybir

    f32 = mybir.dt.float32
    f32r = mybir.dt.float32r
    i32 = mybir.dt.int32
    AF = mybir.ActivationFunctionType
    OP = mybir.AluOpType
    AX = mybir.AxisListType

    nc = bacc.Bacc(
        "TRN2",
        target_bir_lowering=False,
        debug=False,
        num_devices=NCORES if with_collective else 1,
    )

    x_dram = nc.dram_tensor("x", [ROWS, POS], f32, kind="ExternalInput")
    mz_dram = nc.dram_tensor("mz", [128, 128], f32, kind="ExternalInput")
    ms_dram = nc.dram_tensor("ms", [128, 128], f32, kind="ExternalInput")
    out_dram = nc.dram_tensor("out", [ROWS, POS], f32, kind="ExternalOutput")

    with tile.TileContext(nc) as tc:
        with (
            tc.tile_pool(name="pers", bufs=1) as pers,
            tc.tile_pool(name="wk", bufs=3) as wk,
            tc.tile_pool(name="psum", bufs=2, space="PSUM") as psum,
            tc.tile_pool(name="dram", bufs=1, space="DRAM") as dram,
        ):
            xt = [
                pers.tile([128, POS], f32r, tag="x0", name="x0"),
                pers.tile([128, POS], f32r, tag="x1", name="x1"),
            ]
            xf = [t.bitcast(f32) for t in xt]
            mz_sb = pers.tile([128, 128], f32r, tag="mz", name="mz")
            ms_sb = pers.tile([128, 128], f32r, tag="ms", name="ms")
            nc.sync.dma_start(mz_sb[:], mz_dram[:, :].bitcast(f32r))
            nc.sync.dma_start(ms_sb[:], ms_dram[:, :].bitcast(f32r))

            NB = POS // SC                   # bn_stats chunks per tile
            bnb = [
                pers.tile([128, NB * (SC // 512) * 6], f32, tag=f"bnb{k}", name=f"bnb{k}")
                for k in range(2)
            ]
            bna = [
                pers.tile([128, 2], f32, tag=f"bna{k}", name=f"bna{k}")
                for k in range(2)
            ]
            half_sb = pers.tile([128, 1], f32, tag="half", name="half")
            nc.vector.memset(half_sb[:], 0.5)

            # Phase 1: load + per-(b,c) stats; one small AllReduce per tile so
            # tile0's stats fly while tile1 is still loading.
            sq = []
            negmusq = []
            # taper the last loads so the final bn_stats tail (which gates
            # the AllReduce) is short: 3x4096 + 1024*3 + 512*2
            load_sizes = [4096, 4096, 4096, 1024, 1024, 1024, 512, 512]
            assert sum(load_sizes) == POS
            for k in range(2):
                rows = slice(k * 128, (k + 1) * 128)
                off = 0
                for ls in load_sizes:
                    cs = slice(off, off + ls)
                    nc.sync.dma_start(xt[k][:, cs], x_dram[rows, cs].bitcast(f32r))
                    for b in range(ls // 512):
                        j = off // 512 + b
                        nc.vector.bn_stats(
                            bnb[k][:, j * 6 : (j + 1) * 6],
                            xf[k][:, j * 512 : (j + 1) * 512],
                        )
                    off += ls
                nc.vector.bn_aggr(bna[k][:], bnb[k][:])
                # S1 = mean*NLOC ; S2 = (var + mean^2)*NLOC
                stats2 = pers.tile([128, 2], f32, tag=f"stats2_{k}", name=f"stats2_{k}")
                mean = bna[k][:, 0:1]
                var = bna[k][:, 1:2]
                m2 = pers.tile([128, 1], f32, tag=f"m2_{k}", name=f"m2_{k}")
                nc.vector.tensor_mul(m2[:], mean, mean)
                nc.vector.tensor_add(m2[:], m2[:], var)
                nc.vector.tensor_scalar_mul(stats2[:, 0:1], mean, NLOC)
                nc.vector.tensor_scalar_mul(stats2[:, 1:2], m2[:], NLOC)

                cc_in = dram.tile([128, 2], f32, tag=f"cc_in{k}", name=f"cc_in{k}")
                cc_out = dram.tile([128, 2], f32, tag=f"cc_out{k}", name=f"cc_out{k}")
                gstats = pers.tile([128, 2], f32, tag=f"gstats{k}", name=f"gstats{k}")
                nc.sync.dma_start(cc_in[:], stats2[:])
                if with_collective:
                    nc.gpsimd.collective_compute(
                        "AllReduce",
                        OP.add,
                        replica_groups=[list(range(NCORES))],
                        ins=[cc_in.opt()],
                        outs=[cc_out.opt()],
                    )
                else:
                    # single-core timing variant: pass-through instead of AR
                    nc.sync.dma_start(cc_out[:], cc_in[:])
                nc.sync.dma_start(gstats[:], cc_out[:])

                # Per-partition scalars:
                # mu = S1/NG ; sumd2 = S2 - S1^2/NG ; den = 4*(sumd2/n + lam)
                # sq = 1/sqrt(den) (Quake seed + 2 Newton) ; nms = -mu*sq
                S1 = gstats[:, 0:1]
                S2 = gstats[:, 1:2]
                t1 = pers.tile([128, 1], f32, tag=f"t1_{k}", name=f"t1_{k}")
                den = pers.tile([128, 1], f32, tag=f"den_{k}", name=f"den_{k}")
                s = pers.tile([128, 1], f32, tag=f"s_{k}", name=f"s_{k}")
                u = pers.tile([128, 1], f32, tag=f"u_{k}", name=f"u_{k}")
                v = pers.tile([128, 1], f32, tag=f"v_{k}", name=f"v_{k}")
                nms = pers.tile([128, 1], f32, tag=f"nms_{k}", name=f"nms_{k}")
                mu = pers.tile([128, 1], f32, tag=f"mu_{k}", name=f"mu_{k}")

                nc.vector.tensor_mul(t1[:], S1, S1)
                # den' = S2 - S1^2/NG  (then affine to denom)
                nc.vector.scalar_tensor_tensor(
                    den[:], t1[:], -1.0 / NG, S2, op0=OP.mult, op1=OP.add
                )
                nc.vector.tensor_scalar(
                    den[:], den[:], 4.0 / NSIMAM, 4.0 * LAM, op0=OP.mult, op1=OP.add
                )
                # Quake rsqrt seed on int32 view: i = 0x5f3759df - (i >> 1)
                nc.vector.tensor_scalar(
                    s[:].bitcast(i32),
                    den[:].bitcast(i32),
                    1,
                    None,
                    op0=OP.arith_shift_right,
                )
                nc.vector.tensor_scalar(
                    s[:].bitcast(i32),
                    s[:].bitcast(i32),
                    -1,
                    0x5F3759DF,
                    op0=OP.mult,
                    op1=OP.add,
                )
                # two Newton steps: s = s*(1.5 - 0.5*den*s^2)
                for _ in range(2):
                    nc.vector.tensor_mul(u[:], s[:], s[:])
                    nc.vector.tensor_mul(u[:], u[:], den[:])
                    nc.vector.tensor_scalar(
                        v[:], u[:], -0.5, 1.5, op0=OP.mult, op1=OP.add
                    )
                    nc.vector.tensor_mul(s[:], s[:], v[:])
                nc.vector.tensor_scalar_mul(mu[:], S1, 1.0 / NG)
                nc.vector.scalar_tensor_tensor(
                    nms[:], mu[:], -1.0, s[:], op0=OP.mult, op1=OP.mult
                )
                sq.append(s)
                negmusq.append(nms)

            # ---------------- Phase 2 ----------------
            # k-major order: all tile-0 chunks first (gated only by the early
            # AllReduce0), then tile-1 — avoids head-of-line stalls on the
            # later AllReduce1 in the strict-FIFO engine queues.  sf is
            # recomputed per k (TensorE has ~4x slack).
            split = 0
            for k in range(2):
                rows = slice(k * 128, (k + 1) * 128)
                for j10 in range(POS // AC):
                    ssl = slice(j10 * AC, (j10 + 1) * AC)
                    zf = {}
                    sfl = {}
                    for h in range(AC // FC):
                        j = (AC // FC) * j10 + h
                        fsl = slice(j * FC, (j + 1) * FC)
                        sfp = psum.tile([128, FC], f32, tag="sf", name="sf", bufs=4)
                        nc.tensor.matmul(
                            sfp[:], lhsT=ms_sb[:], rhs=xt[0][:, fsl],
                            start=True, stop=False,
                        )
                        nc.tensor.matmul(
                            sfp[:], lhsT=ms_sb[:], rhs=xt[1][:, fsl],
                            start=False, stop=True,
                        )
                        sfl[h] = sfp
                        zp = psum.tile([128, FC], f32, tag="zf", name="zf", bufs=4)
                        nc.tensor.matmul(
                            zp[:], lhsT=mz_sb[:], rhs=xt[k][:, fsl],
                            start=True, stop=True,
                        )
                        zf[h] = zp

                    # U = invd*(x-mu)^2 = y - 0.5
                    U = wk.tile([128, AC], f32, tag="U", name="U")
                    nc.scalar.activation(
                        U[:], xf[k][:, ssl], AF.Square,
                        bias=negmusq[k][:], scale=sq[k][:],
                    )
                    P = wk.tile([128, AC], f32, tag="P", name="P")
                    nc.scalar.activation(P[:], U[:], AF.Sigmoid, bias=half_sb[:])
                    W2 = wk.tile([128, AC], f32, tag="W2", name="W2")
                    for h in range(AC // FC):
                        hs = slice(h * FC, (h + 1) * FC)
                        w1 = wk.tile([128, FC], f32, tag="w1", name="w1", bufs=2)
                        # w1 = (U + 0.5) * zf = y*z
                        nc.vector.scalar_tensor_tensor(
                            w1[:], U[:, hs], 0.5, zf[h][:],
                            op0=OP.add, op1=OP.mult,
                        )
                        # W2 = y*z*s
                        nc.vector.tensor_mul(W2[:, hs], w1[:], sfl[h][:])
                    Q = wk.tile([128, AC], f32, tag="Q", name="Q")
                    nc.scalar.activation(Q[:], W2[:], AF.Sigmoid)
                    last = k == 1 and j10 == POS // AC - 1
                    O1 = wk.tile([128, AC], f32, tag="O1", name="O1")
                    # the very last chunk's muls run on the faster VectorE
                    # (it is otherwise finished) to shorten the drain chain
                    o1eng = nc.vector if last else nc.gpsimd
                    o1eng.tensor_mul(O1[:], xf[k][:, ssl], P[:])
                    R = wk.tile([128, AC], f32, tag="R", name="R")
                    # final muls split evenly between VectorE and GPSIMD
                    eng = nc.vector if (split % 2 == 0 or last) else nc.gpsimd
                    split += 1
                    eng.tensor_mul(R[:], O1[:], Q[:])
                    nc.sync.dma_start(out_dram[rows, ssl], R[:])

    nc.compile()
    return nc


def _get_nc():
    if "nc" not in _CACHE:
        _CACHE["nc"] = _build()
    return _CACHE["nc"]


def _run(x, **kwargs):
    from concourse.bass_utils import run_bass_kernel_spmd

    x = np.ascontiguousarray(np.asarray(x, dtype=np.float32))
    assert x.shape == (B, C, T, H, W), x.shape
    mz, ms = _mats()
    nc = _get_nc()

    in_maps = []
    for i in range(NCORES):
        xi = np.ascontiguousarray(x[:, :, i * TLOC : (i + 1) * TLOC]).reshape(ROWS, POS)
        in_maps.append({"x": xi, "mz": mz, "ms": ms})

    try:
        res = run_bass_kernel_spmd(nc, in_maps, core_ids=list(range(NCORES)), **kwargs)
    except Exception:
        # one retry: transient NRT device faults on first touch were observed
        res = run_bass_kernel_spmd(nc, in_maps, core_ids=list(range(NCORES)), **kwargs)

    out = np.empty((B, C, T, H, W), dtype=np.float32)
    for i in range(NCORES):
        out[:, :, i * TLOC : (i + 1) * TLOC] = (
            res.results[i]["out"].reshape(B, C, TLOC, H, W)
        )
    return out, res


def kernel(x):
    out, _ = _run(x)
    return out


# revision 49
# speedup vs baseline: 1.0069x; 1.0069x over previous
"""Trainium2 Bass kernel for nn_PfAAMLayer (SimAM-style attention layer).

Reference math (x: [b=8, c=32, t=32, h=64, w=64] fp32):
    z  = mean(x, axis=1)                      # channel-mean gate  [b,1,t,h,w]
    s  = mean(x, axis=0)                      # batch-mean gate    [1,c,t,h,w]
    mu = mean(x, axis=(2,3,4))                # per-(b,c) mean
    d2 = (x - mu)^2
    denom = 4*(sum(d2, (2,3,4))/(h*w-1) + 1e-4)
    y  = d2/denom + 0.5
    out = x * sigmoid(y) * sigmoid(y*z*s)

Sharding: split t (axis 2) across 8 cores -> z and s are core-local;
only the per-(b,c) scalars sum(x), sum(x^2) need a 512-float AllReduce.

Per-core layout: x_local viewed as [256 (b,c) rows, 16384 (t_loc,h,w) cols],
held SBUF-resident as two [128, 16384] tiles (dtype float32r so the
TensorEngine consumes them at full rate; every other engine bitcasts to
fp32).  z/s broadcasts are produced on TensorE with 0/1 averaging matrices,
per-(b,c) stats come from one bn_stats pass on VectorE, the squares and
sigmoids run on ScalarE with per-partition scale/bias, and the remaining
elementwise multiplies are split between VectorE and GPSIMD.
"""

import sys

sys.path.insert(0, "/opt/trn_rl_repo")

import numpy as np

B, C, T, H, W = 8, 32, 32, 64, 64
NCORES = 8
TLOC = T // NCORES           # 4
POS = TLOC * H * W           # 16384 positions per core
NLOC = float(POS)
ROWS = B * C                 # 256 (b,c) rows -> 2 tiles of 128
NG = float(T * H * W)        # 131072, global per-(b,c) element count
NSIMAM = float(W * H - 1)    # 4095 (reference quirk: h*w-1, not n-1)
LAM = 1e-4

SC = 4096                    # load / bn_stats chunk (free dim)
AC = 1024                    # phase-2 ACT / output chunk
FC = 512                     # PSUM / fine chunk (one PSUM bank of fp32)

_CACHE = {}


def _mats():
    """Averaging matrices: out[m,n] = sum_p M[p,m] * x[p,n].

    Mz: mean over c within the b-group (32 partitions/group).
    Ms: mean over b (rows p = b*32 + c; same c = same p mod 32); applied to
        both 128-row tiles with PSUM accumulation -> mean over all 8 b.
    """
    p = np.arange(128)
    m = np.arange(128)
    mz = np.where(p[:, None] // 32 == m[None, :] // 32, np.float32(1.0 / 32), np.float32(0.0))
    ms = np.where(p[:, None] % 32 == m[None, :] % 32, np.float32(1.0 / 8), np.float32(0.0))
    return np.ascontiguousarray(mz, np.float32), np.ascontiguousarray(ms, np.float32)


def _build(with_collective=True):
    import concourse.bass as bass
    import concourse.bacc as bacc
    import concourse.tile as tile
    import concourse.mybir as mybir

    f32 = mybir.dt.float32
    f32r = mybir.dt.float32r
    i32 = mybir.dt.int32
    AF = mybir.ActivationFunctionType
    OP = mybir.AluOpType
    AX = mybir.AxisListType

    nc = bacc.Bacc(
        "TRN2",
        target_bir_lowering=False,
        debug=False,
        num_devices=NCORES if with_collective else 1,
    )

    x_dram = nc.dram_tensor("x", [ROWS, POS], f32, kind="ExternalInput")
    mz_dram = nc.dram_tensor("mz", [128, 128], f32, kind="ExternalInput")
    ms_dram = nc.dram_tensor("ms", [128, 128], f32, kind="ExternalInput")
    out_dram = nc.dram_tensor("out", [ROWS, POS], f32, kind="ExternalOutput")

    with tile.TileContext(nc) as tc:
        with (
            tc.tile_pool(name="pers", bufs=1) as pers,
            tc.tile_pool(name="wk", bufs=3) as wk,
            tc.tile_pool(name="psum", bufs=2, space="PSUM") as psum,
            tc.tile_pool(name="dram", bufs=1, space="DRAM") as dram,
        ):
            xt = [
                pers.tile([128, POS], f32r, tag="x0", name="x0"),
                pers.tile([128, POS], f32r, tag="x1", name="x1"),
            ]
            xf = [t.bitcast(f32) for t in xt]
            mz_sb = pers.tile([128, 128], f32r, tag="mz", name="mz")
            ms_sb = pers.tile([128, 128], f32r, tag="ms", name="ms")
            nc.sync.dma_start(mz_sb[:], mz_dram[:, :].bitcast(f32r))
            nc.sync.dma_start(ms_sb[:], ms_dram[:, :].bitcast(f32r))

            NB = POS // SC                   # bn_stats chunks per tile
            bnb = [
                pers.tile([128, NB * (SC // 512) * 6], f32, tag=f"bnb{k}", name=f"bnb{k}")
                for k in range(2)
            ]
            bna = [
                pers.tile([128, 2], f32, tag=f"bna{k}", name=f"bna{k}")
                for k in range(2)
            ]
            half_sb = pers.tile([128, 1], f32, tag="half", name="half")
            nc.vector.memset(half_sb[:], 0.5)

            # Phase 1: load + per-(b,c) stats; one small AllReduce per tile so
            # tile0's stats fly while tile1 is still loading.
            sq = []
            negmusq = []
            # taper the last loads so the final bn_stats tail (which gates
            # the AllReduce) is short: 3x4096 + 1024*3 + 512*2
            load_sizes = [4096, 4096, 4096, 1024, 1024, 1024, 512, 512]
            assert sum(load_sizes) == POS
            for k in range(2):
                rows = slice(k * 128, (k + 1) * 128)
                off = 0
                for ls in load_sizes:
                    cs = slice(off, off + ls)
                    nc.sync.dma_start(xt[k][:, cs], x_dram[rows, cs].bitcast(f32r))
                    for b in range(ls // 512):
                        j = off // 512 + b
                        nc.vector.bn_stats(
                            bnb[k][:, j * 6 : (j + 1) * 6],
                            xf[k][:, j * 512 : (j + 1) * 512],
                        )
                    off += ls
                nc.vector.bn_aggr(bna[k][:], bnb[k][:])
                # S1 = mean*NLOC ; S2 = (var + mean^2)*NLOC
                stats2 = pers.tile([128, 2], f32, tag=f"stats2_{k}", name=f"stats2_{k}")
                mean = bna[k][:, 0:1]
                var = bna[k][:, 1:2]
                m2 = pers.tile([128, 1], f32, tag=f"m2_{k}", name=f"m2_{k}")
                nc.vector.tensor_mul(m2[:], mean, mean)
                nc.vector.tensor_add(m2[:], m2[:], var)
                nc.vector.tensor_scalar_mul(stats2[:, 0:1], mean, NLOC)
                nc.vector.tensor_scalar_mul(stats2[:, 1:2], m2[:], NLOC)

                cc_in = dram.tile([128, 2], f32, tag=f"cc_in{k}", name=f"cc_in{k}")
                cc_out = dram.tile([128, 2], f32, tag=f"cc_out{k}", name=f"cc_out{k}")
                gstats = pers.tile([128, 2], f32, tag=f"gstats{k}", name=f"gstats{k}")
                nc.sync.dma_start(cc_in[:], stats2[:])
                if with_collective:
                    nc.gpsimd.collective_compute(
                        "AllReduce",
                        OP.add,
                        replica_groups=[list(range(NCORES))],
                        ins=[cc_in.opt()],
                        outs=[cc_out.opt()],
                    )
                else:
                    # single-core timing variant: pass-through instead of AR
                    nc.sync.dma_start(cc_out[:], cc_in[:])
                nc.sync.dma_start(gstats[:], cc_out[:])

                # Per-partition scalars:
                # mu = S1/NG ; sumd2 = S2 - S1^2/NG ; den = 4*(sumd2/n + lam)
                # sq = 1/sqrt(den) (Quake seed + 2 Newton) ; nms = -mu*sq
                S1 = gstats[:, 0:1]
                S2 = gstats[:, 1:2]
                t1 = pers.tile([128, 1], f32, tag=f"t1_{k}", name=f"t1_{k}")
                den = pers.tile([128, 1], f32, tag=f"den_{k}", name=f"den_{k}")
                s = pers.tile([128, 1], f32, tag=f"s_{k}", name=f"s_{k}")
                u = pers.tile([128, 1], f32, tag=f"u_{k}", name=f"u_{k}")
                v = pers.tile([128, 1], f32, tag=f"v_{k}", name=f"v_{k}")
                nms = pers.tile([128, 1], f32, tag=f"nms_{k}", name=f"nms_{k}")
                mu = pers.tile([128, 1], f32, tag=f"mu_{k}", name=f"mu_{k}")

                nc.vector.tensor_mul(t1[:], S1, S1)
                # den' = S2 - S1^2/NG  (then affine to denom)
                nc.vector.scalar_tensor_tensor(
                    den[:], t1[:], -1.0 / NG, S2, op0=OP.mult, op1=OP.add
                )
                nc.vector.tensor_scalar(
                    den[:], den[:], 4.0 / NSIMAM, 4.0 * LAM, op0=OP.mult, op1=OP.add
                )
                # Quake rsqrt seed on int32 view: i = 0x5f3759df - (i >> 1)
                nc.vector.tensor_scalar(
                    s[:].bitcast(i32),
                    den[:].bitcast(i32),
                    1,
                    None,
                    op0=OP.arith_shift_right,
                )
                nc.vector.tensor_scalar(
                    s[:].bitcast(i32),
                    s[:].bitcast(i32),
                    -1,
                    0x5F3759DF,
                    op0=OP.mult,
                    op1=OP.add,
                )
                # two Newton steps: s = s*(1.5 - 0.5*den*s^2)
                for _ in range(2):
                    nc.vector.tensor_mul(u[:], s[:], s[:])
                    nc.vector.tensor_mul(u[:], u[:], den[:])
                    nc.vector.tensor_scalar(
                        v[:], u[:], -0.5, 1.5, op0=OP.mult, op1=OP.add
                    )
                    nc.vector.tensor_mul(s[:], s[:], v[:])
                nc.vector.tensor_scalar_mul(mu[:], S1, 1.0 / NG)
                nc.vector.scalar_tensor_tensor(
                    nms[:], mu[:], -1.0, s[:], op0=OP.mult, op1=OP.mult
                )
                sq.append(s)
                negmusq.append(nms)

            # ---------------- Phase 2 ----------------
            # k-major order: all tile-0 chunks first (gated only by the early
            # AllReduce0), then tile-1 — avoids head-of-line stalls on the
            # later AllReduce1 in the strict-FIFO engine queues.  sf is
            # recomputed per k (TensorE has ~4x slack).
            split = 0
            for k in range(2):
                rows = slice(k * 128, (k + 1) * 128)
                for j10 in range(POS // AC):
                    ssl = slice(j10 * AC, (j10 + 1) * AC)
                    zf = {}
                    sfl = {}
                    for h in range(AC // FC):
                        j = (AC // FC) * j10 + h
                        fsl = slice(j * FC, (j + 1) * FC)
                        sfp = psum.tile([128, FC], f32, tag="sf", name="sf", bufs=4)
                        nc.tensor.matmul(
                            sfp[:], lhsT=ms_sb[:], rhs=xt[0][:, fsl],
                            start=True, stop=False,
                        )
                        nc.tensor.matmul(
                            sfp[:], lhsT=ms_sb[:], rhs=xt[1][:, fsl],
                            start=False, stop=True,
                        )
                        sfl[h] = sfp
                        zp = psum.tile([128, FC], f32, tag="zf", name="zf", bufs=4)
                        nc.tensor.matmul(
                            zp[:], lhsT=mz_sb[:], rhs=xt[k][:, fsl],
                            start=True, stop=True,
                        )
                        zf[h] = zp

                    # U = invd*(x-mu)^2 = y - 0.5
                    U = wk.tile([128, AC], f32, tag="U", name="U")
                    nc.scalar.activation(
                        U[:], xf[k][:, ssl], AF.Square,
                        bias=negmusq[k][:], scale=sq[k][:],
                    )
                    P = wk.tile([128, AC], f32, tag="P", name="P")
                    nc.scalar.activation(P[:], U[:], AF.Sigmoid, bias=half_sb[:])
                    W2 = wk.tile([128, AC], f32, tag="W2", name="W2")
                    for h in range(AC // FC):
                        hs = slice(h * FC, (h + 1) * FC)
                        w1 = wk.tile([128, FC], f32, tag="w1", name="w1", bufs=2)
                        # w1 = (U + 0.5) * zf = y*z
                        nc.vector.scalar_tensor_tensor(
                            w1[:], U[:, hs], 0.5, zf[h][:],
                            op0=OP.add, op1=OP.mult,
                        )
                        # W2 = y*z*s
                        nc.vector.tensor_mul(W2[:, hs], w1[:], sfl[h][:])
                    Q = wk.tile([128, AC], f32, tag="Q", name="Q")
                    nc.scalar.activation(Q[:], W2[:], AF.Sigmoid)
                    last = k == 1 and j10 == POS // AC - 1
                    O1 = wk.tile([128, AC], f32, tag="O1", name="O1")
                    # the very last chunk's muls run on the faster VectorE
                    # (it is otherwise finished) to shorten the drain chain
                    o1eng = nc.vector if last else nc.gpsimd
                    o1eng.tensor_mul(O1[:], xf[k][:, ssl], P[:])
                    R = wk.tile([128, AC], f32, tag="R", name="R")
                    # final muls split evenly between VectorE and GPSIMD
                    eng = nc.vector if ((split % 2 == 0 and split != 30) or last) else nc.gpsimd
                    split += 1
                    eng.tensor_mul(R[:], O1[:], Q[:])
                    nc.sync.dma_start(out_dram[rows, ssl], R[:])

    nc.compile()
    return nc


def _get_nc():
    if "nc" not in _CACHE:
        _CACHE["nc"] = _build()
    return _CACHE["nc"]


def _run(x, **kwargs):
    from concourse.bass_utils import run_bass_kernel_spmd

    x = np.ascontiguousarray(np.asarray(x, dtype=np.float32))
    assert x.shape == (B, C, T, H, W), x.shape
    mz, ms = _mats()
    nc = _get_nc()

    in_maps = []
    for i in range(NCORES):
        xi = np.ascontiguousarray(x[:, :, i * TLOC : (i + 1) * TLOC]).reshape(ROWS, POS)
        in_maps.append({"x": xi, "mz": mz, "ms": ms})

    try:
        res = run_bass_kernel_spmd(nc, in_maps, core_ids=list(range(NCORES)), **kwargs)
    except Exception:
        # one retry: transient NRT device faults on first touch were observed
        res = run_bass_kernel_spmd(nc, in_maps, core_ids=list(range(NCORES)), **kwargs)

    out = np.empty((B, C, T, H, W), dtype=np.float32)
    for i in range(NCORES):
        out[:, :, i * TLOC : (i + 1) * TLOC] = (
            res.results[i]["out"].reshape(B, C, TLOC, H, W)
        )
    return out, res


def kernel(x):
    out, _ = _run(x)
    return out
